# revision 14
# baseline (speedup 1.0000x reference)
"""DCRNN cell (diffusion conv GRU step, K=3) on 8 trn2 NeuronCores.

Sharding: nodes are balance-partitioned (snake by degree) into 8 cores x SB
blocks of 128 slots.  Each core owns the edges whose destination falls in its
node range (per direction), does gather (indirect DMA) + one-hot-selector
matmul scatter into PSUM for both diffusion hops, with one AllGather halo
exchange of the scaled hop-1 results between hops.  Gates/head are dense
matmuls on the owned slice.

Since H0 = 0 in the reference, only the first IN_CH rows of the gate weights
matter and the R gate has no effect on the output; this kernel exploits both.
"""

import os
import sys

for _p in ("/opt/pypackages", "/opt/trn_rl_repo"):
    if _p not in sys.path:
        sys.path.insert(0, _p)

from contextlib import ExitStack

import numpy as np

import concourse.bass as bass
import concourse.mybir as mybir
import concourse.tile as tile
from concourse import bacc
from concourse.bass import AP
from concourse.library_config import mlp as mlp_library
from concourse.masks import make_identity

F16 = mybir.dt.float16
F32 = mybir.dt.float32
I16 = mybir.dt.int16
I32 = mybir.dt.int32

N_CORES = 8
P = 128  # partitions / block size
WG = 8  # dst blocks per gather window


def _ceil_div(a, b):
    return -(-a // b)


# ----------------------------------------------------------------------------
# Host-side prep: permutation, edge bucketing, padded layouts (index work only)
# ----------------------------------------------------------------------------


class HostPlan:
    pass


def host_prep(x, edge_index, edge_weight):
    n, IN = x.shape
    row = edge_index[0].astype(np.int64)
    col = edge_index[1].astype(np.int64)
    w = edge_weight.astype(np.float32)
    E = row.shape[0]

    SB = _ceil_div(n, N_CORES * P)  # blocks per core
    NS = N_CORES * SB * P  # total node slots
    SBB = SB * P  # slots per core
    HALF = NS // 2
    assert HALF <= 32768, "int16 gather index range exceeded"

    # --- snake assignment of nodes to (core, block) bins, balancing degree ---
    cnt = np.bincount(row, minlength=n) + np.bincount(col, minlength=n)
    order = np.argsort(-cnt, kind="stable")
    nbins = N_CORES * SB
    idx = np.arange(n)
    rnd = idx // nbins
    pos = idx % nbins
    bins = np.where(rnd % 2 == 0, pos, nbins - 1 - pos)
    g_sorted = bins * P + rnd  # slot for order[i]
    node2g = np.empty(n, np.int64)
    node2g[order] = g_sorted

    xg = np.zeros((NS, IN), np.float32)
    xg[node2g] = x

    # --- padded per-node weight lists (for degree computation) ---
    def wpad(keys_g):
        o = np.argsort(keys_g, kind="stable")
        ks = keys_g[o]
        ws = w[o]
        starts = np.searchsorted(ks, np.arange(NS))
        r = np.arange(E) - starts[ks]
        cdeg = max(8, int(_ceil_div(int(r.max()) + 1, 4) * 4))
        W = np.zeros((NS, cdeg), np.float16)
        W[ks, r] = ws.astype(np.float16)
        c = np.bincount(ks, minlength=NS)
        W[c == 0, 0] = 1.0  # pad/isolated nodes: deg := 1 (never used)
        return W, cdeg

    wpo, cdeg_o = wpad(node2g[row])
    wpi, cdeg_i = wpad(node2g[col])
    CDEG = max(cdeg_o, cdeg_i)
    if cdeg_o < CDEG:
        wpo = np.pad(wpo, ((0, 0), (0, CDEG - cdeg_o)))
    if cdeg_i < CDEG:
        wpi = np.pad(wpi, ((0, 0), (0, CDEG - cdeg_i)))

    # --- per-direction edge bucketing ---
    wins = [range(s, min(s + WG, SB)) for s in range(0, SB, WG)]

    def make_dir(src_g, dst_g):
        owner = dst_g // SBB
        blk = (dst_g % SBB) // P
        dslot = dst_g % P
        half = src_g // HALF
        idxv = (src_g % HALF).astype(np.int64)
        o = np.lexsort((half, blk, owner))
        owner_s, blk_s, half_s = owner[o], blk[o], half[o]
        idx_s, dslot_s = idxv[o], dslot[o]
        # chunk capacity per (block, half): max over cores
        counts = np.zeros((N_CORES, SB, 2), np.int64)
        np.add.at(counts, (owner_s, blk_s, half_s), 1)
        C = _ceil_div(counts, P).max(axis=0)  # [SB, 2]
        # flat chunk layout: for win: for half: for blk in win
        start_chunk = np.zeros((SB, 2), np.int64)
        ct = 0
        for wi in wins:
            for h in (0, 1):
                for b in wi:
                    start_chunk[b, h] = ct
                    ct += C[b, h]
        NCH = ct
        EF = NCH * P
        # scatter edges into flat arrays
        gk = (owner_s * SB + blk_s) * 2 + half_s
        gstart = np.searchsorted(gk, np.arange(N_CORES * SB * 2))
        r = np.arange(E) - gstart[gk]
        posf = start_chunk[blk_s, half_s] * P + r
        idx_flat = np.zeros((N_CORES, EF), np.int16)
        d_flat = np.full((N_CORES, EF), -1.0, np.float16)
        idx_flat[owner_s, posf] = idx_s.astype(np.int16)
        d_flat[owner_s, posf] = dslot_s.astype(np.float16)
        # device layouts
        idx_t = np.ascontiguousarray(
            np.tile(idx_flat.reshape(N_CORES, EF // 16, 16).transpose(0, 2, 1), (1, 8, 1))
        )  # [N_CORES, 128, EF//16]
        d_t = np.ascontiguousarray(d_flat.reshape(N_CORES, EF // P, P).transpose(0, 2, 1))
        d = HostPlan()
        d.C = C
        d.start_chunk = start_chunk
        d.NCH = NCH
        d.EF = EF
        d.idx_t = idx_t
        d.d_t = d_t
        return d

    fwd = make_dir(node2g[row], node2g[col])
    rev = make_dir(node2g[col], node2g[row])

    pl = HostPlan()
    pl.n, pl.IN, pl.SB, pl.NS, pl.SBB, pl.HALF, pl.CDEG = n, IN, SB, NS, SBB, HALF, CDEG
    pl.wins = wins
    pl.node2g = node2g
    pl.xg = xg
    pl.wpo, pl.wpi = wpo, wpi
    pl.fwd, pl.rev = fwd, rev
    return pl


# ----------------------------------------------------------------------------
# Device program
# ----------------------------------------------------------------------------


def build_program(pl, OUT, OSZ):
    """OUT: gate output channels (128); OSZ: final head size (12)."""
    IN, SB, NS, SBB, HALF, CDEG = pl.IN, pl.SB, pl.NS, pl.SBB, pl.HALF, pl.CDEG
    NBLK4 = NS // (4 * P)  # 4-block groups over all nodes

    nc = bacc.Bacc(
        "TRN2", target_bir_lowering=False, debug=False, num_devices=N_CORES,
        enable_asserts=False,
    )

    # ---- I/O ----
    xg_d = nc.dram_tensor("xg", [NS, IN], F32, kind="ExternalInput").ap()
    wpo_d = nc.dram_tensor("wpo", [NS, CDEG], F16, kind="ExternalInput").ap()
    wpi_d = nc.dram_tensor("wpi", [NS, CDEG], F16, kind="ExternalInput").ap()
    xm_d = nc.dram_tensor("xm", [SBB, IN], F32, kind="ExternalInput").ap()
    wpom_d = nc.dram_tensor("wpom", [SBB, CDEG], F16, kind="ExternalInput").ap()
    wpim_d = nc.dram_tensor("wpim", [SBB, CDEG], F16, kind="ExternalInput").ap()
    fidx_d = nc.dram_tensor("fidx", [P, pl.fwd.EF // 16], I16, kind="ExternalInput").ap()
    fd_d = nc.dram_tensor("fd", [P, pl.fwd.EF // P], F16, kind="ExternalInput").ap()
    ridx_d = nc.dram_tensor("ridx", [P, pl.rev.EF // 16], I16, kind="ExternalInput").ap()
    rd_d = nc.dram_tensor("rd", [P, pl.rev.EF // P], F16, kind="ExternalInput").ap()
    wz_d = nc.dram_tensor("wz", [2, 3, IN, OUT], F32, kind="ExternalInput").ap()
    wh_d = nc.dram_tensor("wh", [2, 3, IN, OUT], F32, kind="ExternalInput").ap()
    bz_d = nc.dram_tensor("bzc", [OUT, 1], F32, kind="ExternalInput").ap()
    bh_d = nc.dram_tensor("bhc", [OUT, 1], F32, kind="ExternalInput").ap()
    wl_d = nc.dram_tensor("wl", [OUT, OSZ], F32, kind="ExternalInput").ap()
    blr_d = nc.dram_tensor("blr", [P, OSZ], F32, kind="ExternalInput").ap()
    out_d = nc.dram_tensor("out", [SBB, OSZ], F32, kind="ExternalOutput").ap()

    xtab = nc.dram_tensor("xtab", [NS, 2 * IN], F16, kind="Internal").ap()
    t1slice = nc.dram_tensor("t1slice", [SBB, 2 * IN], F16, kind="Internal").ap()
    t1tab = nc.dram_tensor(
        "t1tab", [NS, 2 * IN], F16, kind="Internal", addr_space="Shared"
    ).ap()

    with tile.TileContext(nc) as tc:
        sbuf = lambda nm, sh, dt: nc.alloc_sbuf_tensor(nm, sh, dt).ap()

        nc.gpsimd.load_library(mlp_library)

        # ---- static SBUF ----
        iota_i = sbuf("iota_i", [P, P], I32)
        iota16 = sbuf("iota16", [P, P], F16)
        ident = sbuf("ident", [P, P], F32)
        fidx_s = sbuf("fidx_s", [P, pl.fwd.EF // 16], I16)
        fd_s = sbuf("fd_s", [P, pl.fwd.EF // P], F16)
        ridx_s = sbuf("ridx_s", [P, pl.rev.EF // 16], I16)
        rd_s = sbuf("rd_s", [P, pl.rev.EF // P], F16)
        recm_o = sbuf("recm_o", [P, SB], F32)  # my recip deg, block-col packed
        recm_i = sbuf("recm_i", [P, SB], F32)
        X_sb = sbuf("X_sb", [P, SB * IN], F32)  # my x, block-col packed
        To1_sb = sbuf("To1_sb", [P, SB * OUT // 2], F32)  # [128, SB*64]
        Ti1_sb = sbuf("Ti1_sb", [P, SB * IN], F32)
        To2_sb = sbuf("To2_sb", [P, SB * IN], F32)
        Ti2_sb = sbuf("Ti2_sb", [P, SB * IN], F32)
        T1st = sbuf("T1st", [P, SB * 2 * IN], F16)  # staged t1 slice rows
        bz_s = sbuf("bz_s", [OUT, 1], F32)
        bh_s = sbuf("bh_s", [OUT, 1], F32)
        wl_s = sbuf("wl_s", [OUT, OSZ], F16)
        blr_s = sbuf("blr_s", [P, OSZ], F32)
        BZ = [sbuf(f"BZ{i}", [IN, OUT], F16) for i in range(5)]
        BH = [sbuf(f"BH{i}", [IN, OUT], F16) for i in range(5)]
        rec_all = {nm: sbuf(f"rec4_{nm}", [P, 4 * NBLK4], F32) for nm in ("o", "i")}

        nc.gpsimd.iota(iota_i, [[1, P]], channel_multiplier=0)
        nc.vector.tensor_copy(iota16, iota_i)
        make_identity(nc, ident)
        nc.sync.dma_start(fidx_s, fidx_d)
        nc.sync.dma_start(fd_s, fd_d)
        nc.sync.dma_start(ridx_s, ridx_d)
        nc.sync.dma_start(rd_s, rd_d)
        nc.sync.dma_start(bz_s, bz_d)
        nc.sync.dma_start(bh_s, bh_d)
        nc.sync.dma_start(blr_s, blr_d)

        # X_sb: partition p = node slot 128b+p of my range
        nc.sync.dma_start(
            AP(X_sb.tensor, 0, [[SB * IN, P], [IN, SB], [1, IN]]),
            AP(xm_d.tensor, 0, [[IN, P], [P * IN, SB], [1, IN]]),
        )

        # ---- weights prep ----
        with tc.tile_pool(name="wprep", bufs=2) as wp:
            for (src, dst) in ((wz_d, BZ), (wh_d, BH)):
                t0 = wp.tile([IN, OUT], F32, tag="w0")
                t1 = wp.tile([IN, OUT], F32, tag="w1")
                nc.sync.dma_start(t0, src[0, 0])
                nc.sync.dma_start(t1, src[1, 0])
                nc.vector.tensor_tensor(dst[0], t0, t1, op=mybir.AluOpType.add)
                for k, (di, ki) in enumerate(((0, 1), (1, 1), (0, 2), (1, 2))):
                    tk = wp.tile([IN, OUT], F32, tag="wk")
                    nc.sync.dma_start(tk, src[di, ki])
                    nc.vector.tensor_copy(dst[1 + k], tk)
            twl = wp.tile([OUT, OSZ], F32, tag="wl")
            nc.sync.dma_start(twl, wl_d)
            nc.vector.tensor_copy(wl_s, twl)

        # ---- phase 1: degrees + reciprocals ----
        # full-graph recips in 4-rows-per-partition packing (for xtab build)
        with tc.tile_pool(name="degs", bufs=3) as dsb:
            for nm, wsrc in (("o", wpo_d), ("i", wpi_d)):
                rec = rec_all[nm]
                for j in range(NBLK4):
                    wt = dsb.tile([P, 4 * CDEG], F16, tag="wt")
                    nc.sync.dma_start(
                        wt[:],
                        AP(wsrc.tensor, j * 4 * P * CDEG,
                           [[4 * CDEG, P], [1, 4 * CDEG]]),
                    )
                    dg = dsb.tile([P, 4], F32, tag="dg")
                    nc.vector.tensor_reduce(
                        dg[:],
                        wt[:].rearrange("p (j c) -> p j c", c=CDEG),
                        axis=mybir.AxisListType.X, op=mybir.AluOpType.add,
                    )
                    nc.vector.reciprocal(rec[:, 4 * j : 4 * j + 4], dg[:])
            # my recips, block-column packing
            for nm, wsrc, dst in (("o", wpom_d, recm_o), ("i", wpim_d, recm_i)):
                for b0 in range(0, SB, 4):
                    nb = min(4, SB - b0)
                    wt = dsb.tile([P, 4 * CDEG], F16, tag="wtm")
                    nc.sync.dma_start(
                        wt[:, : nb * CDEG],
                        AP(wsrc.tensor, b0 * P * CDEG,
                           [[CDEG, P], [P * CDEG, nb], [1, CDEG]]),
                    )
                    dg = dsb.tile([P, 4], F32, tag="dgm")
                    nc.vector.tensor_reduce(
                        dg[:, :nb],
                        wt[:, : nb * CDEG].rearrange("p (j c) -> p j c", c=CDEG),
                        axis=mybir.AxisListType.X, op=mybir.AluOpType.add,
                    )
                    nc.vector.reciprocal(dst[:, b0 : b0 + nb], dg[:, :nb])

        # ---- phase 2: build xtab = [x/deg_out | x/deg_in] fp16 ----
        with tc.tile_pool(name="xtb", bufs=4) as xp:
            for j in range(NBLK4):
                xt = xp.tile([P, 4 * IN], F32, tag="xin")
                nc.sync.dma_start(
                    xt[:],
                    AP(xg_d.tensor, j * 4 * P * IN, [[4 * IN, P], [1, 4 * IN]]),
                )
                ot = xp.tile([P, 4 * 2 * IN], F16, tag="xout")
                xt3 = xt[:].rearrange("p (j f) -> p j f", f=IN)
                ot3 = ot[:].rearrange("p (j f) -> p j f", f=2 * IN)
                for hx, nm in ((0, "o"), (1, "i")):
                    rb = (
                        rec_all[nm][:, 4 * j : 4 * j + 4]
                        .unsqueeze(2)
                        .to_broadcast([P, 4, IN])
                    )
                    nc.vector.tensor_tensor(
                        ot3[:, :, hx * IN : (hx + 1) * IN], xt3, rb,
                        op=mybir.AluOpType.mult,
                    )
                nc.sync.dma_start(
                    AP(xtab.tensor, j * 4 * P * 2 * IN, [[8 * IN, P], [1, 8 * IN]]),
                    ot[:],
                )

        # ---- gather-scatter hop helper ----
        def run_hop(dirp, idx_s, d_s, table, coloff, post):
            """post(b, psum_ap) consumes the [128, IN] accumulated block."""
            C = dirp.C
            with (
                tc.tile_pool(name="gwin", bufs=2) as gw,
                tc.tile_pool(name="strip", bufs=10) as sp,
                tc.tile_pool(name="pprop", bufs=4, space="PSUM") as pp,
            ):
                strips = {}

                def strip_for(ct):
                    s0 = (ct // 8) * 8
                    if s0 not in strips:
                        st = sp.tile([P, 8 * P], F16, tag="st")
                        nch = min(8, dirp.NCH - s0)
                        nc.vector.tensor_tensor(
                            st[:].rearrange("p (c m) -> p c m", m=P)[:, :nch, :],
                            iota16.unsqueeze(1).to_broadcast([P, nch, P]),
                            d_s[:, s0 : s0 + nch].unsqueeze(2).to_broadcast([P, nch, P]),
                            op=mybir.AluOpType.is_equal,
                        )
                        strips[s0] = st
                    return strips[s0][:, (ct - s0) * P : (ct - s0 + 1) * P]

                for wi in pl.wins:
                    blocks = list(wi)
                    nch_h = [sum(int(C[b, h]) for b in blocks) for h in (0, 1)]
                    nw = nch_h[0] + nch_h[1]
                    if nw == 0:
                        continue
                    wbuf = gw.tile([P, nw, 2 * IN], F16, tag="wb")
                    ct0 = int(dirp.start_chunk[blocks[0], 0])
                    for h in (0, 1):
                        ni = nch_h[h] * P
                        if ni == 0:
                            continue
                        co = 0 if h == 0 else nch_h[0]
                        src = table if h == 0 else table[HALF:]
                        nc.gpsimd.dma_gather(
                            wbuf[:, co : co + nch_h[h], :],
                            src,
                            idx_s[:, (ct0 + co) * 8 : (ct0 + co) * 8 + ni // 16],
                            ni, ni, 2 * IN,
                            single_packet=False,
                        )
                    for b in blocks:
                        tc_chunks = []
                        for h in (0, 1):
                            s = int(dirp.start_chunk[b, h])
                            for k in range(int(C[b, h])):
                                tc_chunks.append(s + k)
                        if not tc_chunks:
                            continue
                        ps = pp.tile([P, IN], F32, tag="ps")
                        for i, ct in enumerate(tc_chunks):
                            lw = ct - ct0  # local chunk within window buffer
                            nc.tensor.matmul(
                                ps[:],
                                lhsT=strip_for(ct),
                                rhs=wbuf[:, lw, coloff : coloff + IN],
                                start=(i == 0),
                                stop=(i == len(tc_chunks) - 1),
                            )
                        post(b, ps)
                    strips.clear()

        # ---- phase 3: hop 1 (+ t1 staging) ----
        def post_hop1(To_sb, recm, hx):
            def post(b, ps):
                nc.scalar.copy(To_sb[:, b * IN : (b + 1) * IN], ps[:])
                nc.vector.tensor_scalar_mul(
                    T1st[:, b * 2 * IN + hx * IN : b * 2 * IN + (hx + 1) * IN],
                    ps[:],
                    recm[:, b : b + 1],
                )
            return post

        run_hop(pl.fwd, fidx_s, fd_s, xtab, 0, post_hop1(To1_sb, recm_o, 0))
        run_hop(pl.rev, ridx_s, rd_s, xtab, IN, post_hop1(Ti1_sb, recm_i, 1))

        # store staged t1 rows; exchange
        nc.sync.dma_start(
            AP(t1slice.tensor, 0, [[2 * IN, P], [P * 2 * IN, SB], [1, 2 * IN]]),
            T1st.rearrange("p (b f) -> p b f", f=2 * IN),
        )
        if os.environ.get("KERNEL_NO_COLL"):
            # debug: skip cross-core exchange (numerically wrong on >1 core)
            for m in range(N_CORES):
                nc.sync.dma_start(
                    t1tab[m * SBB : (m + 1) * SBB], t1slice
                )
        else:
            nc.gpsimd.collective_compute(
                "AllGather",
                mybir.AluOpType.bypass,
                replica_groups=[list(range(N_CORES))],
                ins=[t1slice],
                outs=[t1tab],
            )

        # ---- phase 4: hop 2 ----
        def post_hop2(T2_sb):
            def post(b, ps):
                nc.vector.scalar_tensor_tensor(
                    T2_sb[:, b * IN : (b + 1) * IN],
                    ps[:],
                    2.0,
                    X_sb[:, b * IN : (b + 1) * IN],
                    op0=mybir.AluOpType.mult,
                    op1=mybir.AluOpType.subtract,
                )
            return post

        run_hop(pl.fwd, fidx_s, fd_s, t1tab, 0, post_hop2(To2_sb))
        run_hop(pl.rev, ridx_s, rd_s, t1tab, IN, post_hop2(Ti2_sb))

        # ---- phase 5: gates + head ----
        comps = [X_sb, To1_sb, Ti1_sb, To2_sb, Ti2_sb]
        with (
            tc.tile_pool(name="gts", bufs=8) as gs,
            tc.tile_pool(name="gtp", bufs=2, space="PSUM") as gp,
            tc.tile_pool(name="gtp2", bufs=2, space="PSUM") as gp2,
        ):
            for b0 in range(0, SB, 4):
                nb = min(4, SB - b0)
                compT = []
                for ci, csb in enumerate(comps):
                    pT = gp.tile([IN, 4 * P], F32, tag="pT")
                    for jj in range(nb):
                        nc.tensor.transpose(
                            pT[:, jj * P : (jj + 1) * P],
                            csb[:, (b0 + jj) * IN : (b0 + jj + 1) * IN],
                            ident,
                        )
                    cT = gs.tile([IN, 4 * P], F16, tag=f"cT{ci}")
                    nc.scalar.copy(cT[:, : nb * P], pT[:, : nb * P])
                    compT.append(cT)
                res = {}
                for nm, BW, bias, fn in (
                    ("z", BZ, bz_s, mybir.ActivationFunctionType.Sigmoid),
                    ("h", BH, bh_s, mybir.ActivationFunctionType.Tanh),
                ):
                    pg = gp2.tile([OUT, 4 * P], F32, tag="pg")
                    for ci in range(5):
                        nc.tensor.matmul(
                            pg[:, : nb * P],
                            lhsT=BW[ci],
                            rhs=compT[ci][:, : nb * P],
                            start=(ci == 0),
                            stop=(ci == 4),
                        )
                    act = gs.tile([OUT, 4 * P], F16, tag=f"act{nm}")
                    nc.scalar.activation(act[:, : nb * P], pg[:, : nb * P], fn, bias=bias)
                    res[nm] = act
                omz = gs.tile([OUT, 4 * P], F16, tag="omz")
                nc.scalar.activation(
                    omz[:, : nb * P], res["z"][:, : nb * P],
                    mybir.ActivationFunctionType.Copy, bias=1.0, scale=-1.0,
                )
                hT = gs.tile([OUT, 4 * P], F16, tag="hT")
                nc.vector.tensor_tensor(
                    hT[:, : nb * P], omz[:, : nb * P], res["h"][:, : nb * P],
                    op=mybir.AluOpType.mult,
                )
                hR = gs.tile([OUT, 4 * P], F16, tag="hR")
                nc.scalar.activation(
                    hR[:, : nb * P], hT[:, : nb * P], mybir.ActivationFunctionType.Relu,
                )
                osb = gs.tile([P, 4 * OSZ], F32, tag="osb")
                for jj in range(nb):
                    ph = gp.tile([P, OSZ], F32, tag="ph")
                    nc.tensor.matmul(
                        ph[:], lhsT=hR[:, jj * P : (jj + 1) * P], rhs=wl_s,
                        start=True, stop=True,
                    )
                    nc.vector.tensor_tensor(
                        osb[:, jj * OSZ : (jj + 1) * OSZ], ph[:], blr_s,
                        op=mybir.AluOpType.add,
                    )
                nc.sync.dma_start(
                    AP(out_d.tensor, b0 * P * OSZ,
                       [[OSZ, P], [P * OSZ, nb], [1, OSZ]]),
                    osb[:].rearrange("p (j s) -> p j s", s=OSZ)[:, :nb, :],
                )

    nc.compile()
    return nc


# ----------------------------------------------------------------------------
# Entry
# ----------------------------------------------------------------------------


def _in_maps(pl, Wz, Wh, bz, bh, Wl, bl):
    IN, OUT = pl.IN, Wz.shape[-1]
    shared = dict(
        xg=pl.xg,
        wpo=pl.wpo,
        wpi=pl.wpi,
        wz=np.ascontiguousarray(Wz[:, :, :IN, :], np.float32),
        wh=np.ascontiguousarray(Wh[:, :, :IN, :], np.float32),
        bzc=np.ascontiguousarray(bz.reshape(OUT, 1), np.float32),
        bhc=np.ascontiguousarray(bh.reshape(OUT, 1), np.float32),
        wl=np.ascontiguousarray(Wl, np.float32),
        blr=np.ascontiguousarray(np.tile(bl.reshape(1, -1), (P, 1)), np.float32),
    )
    maps = []
    for m in range(N_CORES):
        sl = slice(m * pl.SBB, (m + 1) * pl.SBB)
        maps.append(
            dict(
                shared,
                xm=np.ascontiguousarray(pl.xg[sl]),
                wpom=np.ascontiguousarray(pl.wpo[sl]),
                wpim=np.ascontiguousarray(pl.wpi[sl]),
                fidx=np.ascontiguousarray(pl.fwd.idx_t[m]),
                fd=np.ascontiguousarray(pl.fwd.d_t[m]),
                ridx=np.ascontiguousarray(pl.rev.idx_t[m]),
                rd=np.ascontiguousarray(pl.rev.d_t[m]),
            )
        )
    return maps


def prepare(x, edge_index, edge_weight, Wz, bz, Wr, br, Wh, bh, Wl, bl):
    x = np.asarray(x, np.float32)
    edge_index = np.asarray(edge_index)
    edge_weight = np.asarray(edge_weight, np.float32)
    pl = host_prep(x, edge_index, edge_weight)
    OUT = np.asarray(Wz).shape[-1]
    OSZ = np.asarray(Wl).shape[-1]
    nc = build_program(pl, OUT, OSZ)
    maps = _in_maps(pl, np.asarray(Wz), np.asarray(Wh), np.asarray(bz),
                    np.asarray(bh), np.asarray(Wl), np.asarray(bl))
    return nc, maps, pl


def kernel(x, edge_index, edge_weight, Wz, bz, Wr, br, Wh, bh, Wl, bl):
    nc, maps, pl = prepare(x, edge_index, edge_weight, Wz, bz, Wr, br,
                           Wh, bh, Wl, bl)

    if os.environ.get("BASS_SIM"):
        from concourse.bass_interp import MultiCoreSim

        sim = MultiCoreSim(nc, num_cores=N_CORES, trace=False)
        for i, core in enumerate(sim.cores.values()):
            for k, v in maps[i].items():
                core.tensor(k)[:] = v
        sim.simulate(check_with_hw=False)
        results = [
            {"out": np.array(core.tensor("out"))} for core in sim.cores.values()
        ]
    else:
        from concourse.bass_utils import run_bass_kernel_spmd

        res = run_bass_kernel_spmd(
            nc, maps, core_ids=list(range(N_CORES)),
            trace=bool(os.environ.get("KERNEL_TRACE")),
        )
        if res.exec_time_ns is not None:
            print(f"HW exec time: {res.exec_time_ns} ns")
        results = res.results

    full = np.concatenate([r["out"] for r in results], axis=0)  # [NS, OSZ]
    return np.ascontiguousarray(full[pl.node2g]).astype(np.float32)
